# revision 1
# baseline (speedup 1.0000x reference)
"""Trainium2 Bass kernel for nn_Encoder (2-layer GCN encoder, graph mean readout).

Math restructuring (exact, up to float reordering):
  Layer 1 (GCNConv + ReLU):  x1 = relu(dis * S + b1),
      S[n] = sum_{e in seg(n)} y[src(e)]  (dst-segments incl. self edge),
      y[m] = dis[m] * (x[m] @ W1),  dis = (deg+1)^-1/2.
  Layer 2 + mean over nodes collapses to a per-node scalar:
      out = (1/N) * (sum_n c[n] * x1[n]) @ W2 + b2,
      c[m] = dis[m] * (sum_{e: src(e)=m} dis[dst(e)] + dis[m]).
So the device kernel only needs: one dense matmul pass producing y (fp16
rows [node, 2*H] in DRAM), one edge-gather + segmented-sum pass (SWDGE
dma_gather + one-hot matmuls into PSUM), and a tiny weighted accumulation.
The final [2,128] @ W2 happens on host (65k FLOPs of the original 13 GFLOP).

Sharding: destination nodes (and their incoming edges) are split across the
8 cores; every core computes the full y table itself (redundant compute is
cheaper than an all-gather at these sizes, and needs no collectives).
Per-core structure is IDENTICAL (SPMD: one program, data-only variation):
destination nodes are bin-packed on host into TILES tiles of 128 psum slots
with a fixed per-src-chunk chunk budget (rotating (5,4,4,4) pattern), so
every gather call / matmul schedule is a compile-time constant.
"""

import sys, os, types
sys.path.insert(0, "/opt/trn_rl_repo")

# antenv.axon_hooks shim (image's antenv stub lacks it); needed for NTFF trace.
if "antenv.axon_hooks" not in sys.modules:
    _hook = [None]
    _m = types.ModuleType("antenv.axon_hooks")
    _m.set_axon_ntff_profile_hook = lambda h: _hook.__setitem__(0, h)
    _m.get_axon_ntff_profile_hook = lambda: _hook[0]
    sys.modules["antenv.axon_hooks"] = _m
    try:
        import antenv
        antenv.axon_hooks = _m
        from trn_agent_boot.trn_boot import _ntff_profile_via_ctypes
        _m.set_axon_ntff_profile_hook(
            _ntff_profile_via_ctypes("/opt/axon/libaxon_pjrt.so"))
    except Exception:
        pass

import numpy as np
from contextlib import ExitStack
from dataclasses import dataclass

import concourse.bacc as bacc
import concourse.bass as bass
import concourse.mybir as mybir
import concourse.tile as tile
from concourse.bass_utils import run_bass_kernel_spmd
from concourse.library_config import mlp

P = 128
H = 128
F_IN = 116
FEXT = F_IN + 8          # one-hot node-type rows appended -> K=124
B = 2
YW = B * H               # 256: y row elements (both batches)


@dataclass(frozen=True)
class Cfg:
    n: int = 100000      # nodes
    ncores: int = 8
    tiles: int = 104     # dst tiles per core (128 slots each)
    chunks: int = 16     # 128-edge chunks per tile (sum over 4 src chunks)
    group: int = 2       # tiles per gather-call group (= psum tiles in flight)
                         # group*chunks/nsc*128 = call_idx must stay <= 1024:
                         # the SWDGE descriptor ring caps a single dma_gather
    nsc: int = 4         # src chunks (int16 gather index reach)

    @property
    def ndst(self):
        return self.n // self.ncores

    @property
    def srcchunk(self):
        return -(-self.n // self.nsc)

    @property
    def npad(self):       # node count padded to phase-1 block (512)
        return -(-self.n // 512) * 512

    @property
    def rot(self):        # rot[r][s]: chunks of tile (t%4==r) in src chunk s
        base, extra = divmod(self.chunks, self.nsc)
        return [[base + (1 if (s - r) % self.nsc < extra else 0)
                 for s in range(self.nsc)] for r in range(self.nsc)]

    @property
    def ngroups(self):
        return self.tiles // self.group

    @property
    def call_chunks(self):  # chunks per gather call = sum_r rot[r][s] (same all s)
        return sum(self.rot[r][0] for r in range(self.group))

    @property
    def call_idx(self):
        return self.call_chunks * P

    @property
    def ncalls(self):
        return self.ngroups * self.nsc

    @property
    def idxcols(self):
        return self.ncalls * (self.call_idx // 16)

    @property
    def nchunks_total(self):
        return self.tiles * self.chunks


CFG = Cfg()

f32 = mybir.dt.float32
f16 = mybir.dt.float16
i16 = mybir.dt.int16


def _build_program(cfg: Cfg, has_b1: bool):
    nc = bacc.Bacc("TRN2")
    xe = nc.dram_tensor("xe", [B, FEXT, cfg.npad], f32, kind="ExternalInput")
    xeo = nc.dram_tensor("xeo", [B, FEXT, cfg.tiles * P], f32,
                         kind="ExternalInput")
    w1e = nc.dram_tensor("w1e", [FEXT, H], f32, kind="ExternalInput")
    idxt = nc.dram_tensor("idxt", [P, cfg.idxcols], i16, kind="ExternalInput")
    dlt = nc.dram_tensor("dlt", [P, cfg.nchunks_total], f16, kind="ExternalInput")
    dcq = nc.dram_tensor("dcq", [P, cfg.tiles], f32, kind="ExternalInput")
    iot = nc.dram_tensor("iot", [P, P], f16, kind="ExternalInput")
    if has_b1:
        disc = nc.dram_tensor("disc", [P, cfg.tiles], f32, kind="ExternalInput")
        cct = nc.dram_tensor("cct", [P, cfg.tiles], f32, kind="ExternalInput")
        b1b = nc.dram_tensor("b1b", [P, YW], f32, kind="ExternalInput")
    y = nc.dram_tensor("y", [cfg.npad, YW], f16, kind="Internal")
    accd = nc.dram_tensor("acc", [P, YW], f32, kind="ExternalOutput")

    nblk = cfg.npad // 512
    rotpre = [[sum(cfg.rot[i][s] for i in range(r)) for s in range(cfg.nsc)]
              for r in range(cfg.group)]

    with tile.TileContext(nc) as tc:
        nc.gpsimd.load_library(mlp)
        with (
            tc.tile_pool(name="const", bufs=1) as cpool,
            tc.tile_pool(name="ph1", bufs=4) as p1pool,
            tc.tile_pool(name="ysb", bufs=3) as ypool,
            tc.tile_pool(name="gat", bufs=4) as gpool,
            tc.tile_pool(name="oh", bufs=8) as ohpool,
            tc.tile_pool(name="x1c", bufs=4) as xpool,
            tc.tile_pool(name="psy", bufs=2, space="PSUM") as psy,
            tc.tile_pool(name="psa", bufs=6, space="PSUM") as psa,
            ExitStack() as ctx,
        ):
            # constants / small preloads
            w1_sb = cpool.tile([FEXT, H], f32, tag="w1")
            nc.sync.dma_start(w1_sb[:], w1e[:])
            iota_sb = cpool.tile([P, P], f16, tag="iota")
            nc.sync.dma_start(iota_sb[:], iot[:])
            dl_sb = cpool.tile([P, cfg.nchunks_total], f16, tag="dl")
            nc.sync.dma_start(dl_sb[:], dlt[:])
            dcq_sb = cpool.tile([P, cfg.tiles], f32, tag="dcq")
            nc.sync.dma_start(dcq_sb[:], dcq[:])
            if has_b1:
                disc_sb = cpool.tile([P, cfg.tiles], f32, tag="disc")
                nc.sync.dma_start(disc_sb[:], disc[:])
                cc_sb = cpool.tile([P, cfg.tiles], f32, tag="cc")
                nc.sync.dma_start(cc_sb[:], cct[:])
                b1_sb = cpool.tile([P, YW], f32, tag="b1b")
                nc.sync.dma_start(b1_sb[:], b1b[:])
            acc_sb = cpool.tile([P, YW], f32, tag="acc")
            nc.vector.memset(acc_sb[:], 0)

            # ---- Phase 1: y[node] = dis*(x @ W1ext), fp16 rows [node, 2*H]
            for blk in range(nblk):
                n0 = blk * 512
                xts = []
                for b in range(B):
                    xt = p1pool.tile([FEXT, 512], f32, tag=f"xt{b}")
                    nc.sync.dma_start(xt[:], xe[b, :, n0:n0 + 512])
                    xts.append(xt)
                for sub in range(4):
                    ysb = ypool.tile([P, YW], f16, tag="ysb")
                    for b in range(B):
                        ps = psy.tile([P, H], f32, tag="psy")
                        nc.tensor.matmul(
                            ps[:],
                            lhsT=xts[b][:, sub * P:(sub + 1) * P],
                            rhs=w1_sb[:], start=True, stop=True)
                        nc.scalar.activation(
                            out=ysb[:, b * H:(b + 1) * H], in_=ps[:],
                            func=mybir.ActivationFunctionType.Copy)
                    r0 = n0 + sub * P
                    nc.sync.dma_start(y[r0:r0 + P, :], ysb[:])

            # ---- Phase 2: gather + segmented one-hot matmul + accumulate
            for g in range(cfg.ngroups):
                pst = [psa.tile([P, YW], f32, tag="psa", name=f"pst{g}_{i}")
                       for i in range(cfg.group)]
                # self-loop inputs: dis^2-scaled own features (see xeo build)
                xos = []
                for b in range(B):
                    xo = p1pool.tile([FEXT, cfg.group * P], f32, tag=f"xo{b}")
                    nc.sync.dma_start(
                        xo[:], xeo[b, :, g * cfg.group * P:(g + 1) * cfg.group * P])
                    xos.append(xo)
                start_mm = [None] * cfg.group
                for s in range(cfg.nsc):
                    call = g * cfg.nsc + s
                    ic0 = call * (cfg.call_idx // 16)
                    idx_sb = gpool.tile([P, cfg.call_idx // 16], i16, tag="idx")
                    nc.sync.dma_start(
                        idx_sb[:], idxt[:, ic0:ic0 + cfg.call_idx // 16])
                    gt = gpool.tile([P, cfg.call_chunks, YW], f16, tag="gt")
                    r0 = s * cfg.srcchunk
                    nc.gpsimd.dma_gather(
                        gt[:], y[r0:r0 + cfg.srcchunk, :], idx_sb[:],
                        cfg.call_idx, cfg.call_idx, YW)
                    for ti in range(cfg.group):
                        t = g * cfg.group + ti
                        k = cfg.rot[ti][s]
                        off = rotpre[ti][s]
                        for j in range(k):
                            # global chunk column for dstloc:
                            gcol = call * cfg.call_chunks + off + j
                            oh = ohpool.tile([P, P], f16, tag="oh")
                            nc.vector.tensor_tensor(
                                out=oh[:],
                                in0=dl_sb[:, gcol:gcol + 1].to_broadcast([P, P]),
                                in1=iota_sb[:],
                                op=mybir.AluOpType.is_equal)
                            # exactly one start=True matmul per psum tile (PSUM
                            # zero-regions are 2KB-wide: start marks the whole
                            # region pending-zero, so it must be unique + first)
                            is_start = (s == 0 and j == 0)
                            mm = nc.tensor.matmul(
                                pst[ti][:], lhsT=oh[:],
                                rhs=gt[:, off + j, :],
                                start=is_start,
                                stop=(s == cfg.nsc - 1 and j == k - 1))
                            if is_start:
                                start_mm[ti] = mm
                                # self-loop term: accumulate xeo @ W1ext into
                                # each batch half, after the start matmul
                                for b in range(B):
                                    sm = nc.tensor.matmul(
                                        pst[ti][:, b * H:(b + 1) * H],
                                        lhsT=xos[b][:, ti * P:(ti + 1) * P],
                                        rhs=w1_sb[:], start=False, stop=False)
                                    bass._add_dep_helper(
                                        sm.ins, start_mm[ti].ins, sync=False,
                                        reason="self-mm after psum start")
                            else:
                                bass._add_dep_helper(
                                    mm.ins, start_mm[ti].ins, sync=False,
                                    reason="accum after psum start")
                for ti in range(cfg.group):
                    t = g * cfg.group + ti
                    x1c = xpool.tile([P, YW], f32, tag="x1c")
                    if not has_b1:
                        # x1c = relu(psum * (dis*c))   (valid since c>0)
                        nc.scalar.activation(
                            out=x1c[:], in_=pst[ti][:],
                            func=mybir.ActivationFunctionType.Relu,
                            bias=0.0, scale=dcq_sb[:, t:t + 1])
                    else:
                        t1 = xpool.tile([P, YW], f32, tag="t1")
                        nc.vector.tensor_scalar(
                            out=t1[:], in0=pst[ti][:],
                            scalar1=disc_sb[:, t:t + 1], scalar2=None,
                            op0=mybir.AluOpType.mult)
                        nc.vector.tensor_tensor(
                            out=t1[:], in0=t1[:], in1=b1_sb[:],
                            op=mybir.AluOpType.add)
                        nc.scalar.activation(
                            out=t1[:], in_=t1[:],
                            func=mybir.ActivationFunctionType.Relu)
                        nc.vector.tensor_scalar(
                            out=x1c[:], in0=t1[:],
                            scalar1=cc_sb[:, t:t + 1], scalar2=None,
                            op0=mybir.AluOpType.mult)
                    nc.vector.tensor_tensor(
                        out=acc_sb[:], in0=acc_sb[:], in1=x1c[:],
                        op=mybir.AluOpType.add)

            nc.sync.dma_start(accd[:], acc_sb[:])

    nc.compile()
    return nc


_PROG_CACHE = {}


def _get_program(cfg: Cfg, has_b1: bool):
    key = (cfg, has_b1)
    if key not in _PROG_CACHE:
        _PROG_CACHE[key] = _build_program(cfg, has_b1)
    return _PROG_CACHE[key]


def _pack_core(cfg: Cfg, core, src, dst, dis_c, n_nodes):
    """Bin-pack this core's dst nodes into tiles; build gather/dstloc/dcq data.

    Returns (idx_w [128, idxcols] i16, dl_w [128, nchunks] f16,
             dcq_w [128, tiles] f32, tile_of, slot_of)."""
    n0 = core * cfg.ndst
    sel = (dst >= n0) & (dst < n0 + cfg.ndst)
    es = src[sel]
    ed = dst[sel]
    # (self edges are handled by the xeown direct matmul, not the gather)
    dl = ed - n0                       # local dst id
    sc = es // cfg.srcchunk            # src chunk of each edge

    cnt = np.bincount(dl * cfg.nsc + sc, minlength=cfg.ndst * cfg.nsc)
    cnt = cnt.reshape(cfg.ndst, cfg.nsc)

    rot = np.array(cfg.rot, dtype=np.int64)          # [4, nsc]
    caps = (rot[np.arange(cfg.tiles) % cfg.nsc] * P).copy()  # [tiles, nsc]
    for s in range(cfg.nsc):
        assert cnt[:, s].sum() <= caps[:, s].sum(), \
            f"core {core}: src chunk {s} demand exceeds capacity"

    order = np.argsort(-cnt.sum(1), kind="stable")
    slots_used = np.zeros(cfg.tiles, dtype=np.int64)
    tile_of = np.full(cfg.ndst, -1, dtype=np.int64)
    slot_of = np.full(cfg.ndst, -1, dtype=np.int64)
    for nloc in order:
        need = cnt[nloc]
        ok = (caps >= need).all(axis=1) & (slots_used < P)
        if not ok.any():
            raise RuntimeError(f"core {core}: bin packing failed for node {nloc}")
        # best fit = feasible tile with most remaining capacity (balances load;
        # with exact slot counts every tile must end up full)
        score = caps.sum(axis=1) + (P - slots_used)
        score[~ok] = -1
        t = int(np.argmax(score))
        tile_of[nloc] = t
        slot_of[nloc] = slots_used[t]
        slots_used[t] += 1
        caps[t] -= need

    # edge stream positions
    et = tile_of[dl]
    eslot = slot_of[dl]
    o = np.lexsort((sc, et))
    et_s, sc_s, slot_s, src_s = et[o], sc[o], eslot[o], es[o]
    ks = et_s * cfg.nsc + sc_s
    counts = np.bincount(ks, minlength=cfg.tiles * cfg.nsc)
    gbase = np.concatenate([[0], np.cumsum(counts)[:-1]])
    rank = np.arange(len(ks)) - gbase[ks]

    # padded stream base for (t, s)
    rotpre = np.zeros((cfg.nsc, cfg.nsc), dtype=np.int64)  # [r, s] prefix
    for r in range(cfg.nsc):
        for s in range(cfg.nsc):
            rotpre[r, s] = sum(cfg.rot[i][s] for i in range(r))
    tt = np.arange(cfg.tiles)
    callno = (tt // cfg.group)[:, None] * cfg.nsc + np.arange(cfg.nsc)[None, :]
    pbase = callno * cfg.call_idx + rotpre[tt % cfg.group] * P  # [tiles, nsc]
    assert (counts.reshape(cfg.tiles, cfg.nsc) <= rot[tt % cfg.nsc] * P).all()

    total = cfg.ncalls * cfg.call_idx
    idx_flat = np.zeros(total, dtype=np.int16)
    dl_flat = np.full(total, 255.0, dtype=np.float16)
    pos = pbase[et_s, sc_s] + rank
    idx_flat[pos] = (src_s - sc_s * cfg.srcchunk).astype(np.int16)
    dl_flat[pos] = slot_s.astype(np.float16)

    ci = cfg.call_idx
    idx_w = idx_flat.reshape(cfg.ncalls, ci // 16, 16).transpose(2, 0, 1)
    idx_w = np.tile(idx_w.reshape(16, -1), (8, 1))           # [128, idxcols]
    dl_w = dl_flat.reshape(cfg.nchunks_total, P).T.copy()    # [128, nchunks]

    dcq_w = np.zeros((P, cfg.tiles), dtype=np.float32)
    dcq_w[slot_of, tile_of] = dis_c[n0:n0 + cfg.ndst]
    return idx_w, dl_w, dcq_w, tile_of, slot_of


def _prepare(cfg: Cfg, node, node_type, edge_index, embed, W1, b1, W2, b2):
    n = cfg.n
    src = edge_index[0].astype(np.int64)
    dst = edge_index[1].astype(np.int64)
    deg = (np.bincount(dst, minlength=n) + 1).astype(np.float32)
    dis = (1.0 / np.sqrt(deg.astype(np.float64))).astype(np.float32)
    s_arr = np.bincount(src, weights=dis[dst].astype(np.float64), minlength=n)
    c = (dis.astype(np.float64) * (s_arr + dis)).astype(np.float32)
    dis_c = (dis.astype(np.float64) * c).astype(np.float32)

    T8 = (embed.astype(np.float64) @ W1[F_IN:, :].astype(np.float64))
    w1e = np.concatenate([W1[:F_IN, :], T8.astype(np.float32)], axis=0)
    w1e = np.ascontiguousarray(w1e, dtype=np.float32)

    xe = np.zeros((B, FEXT, cfg.npad), dtype=np.float32)
    xe[:, :F_IN, :n] = node.transpose(0, 2, 1) * dis[None, None, :]
    oh8 = np.zeros((8, n), dtype=np.float32)
    oh8[node_type.astype(np.int64), np.arange(n)] = dis
    xe[:, F_IN:, :n] = oh8[None]

    iota = np.tile(np.arange(P, dtype=np.float16), (P, 1))

    has_b1 = bool(np.any(b1 != 0))
    in_maps = []
    metas = []
    for core in range(cfg.ncores):
        idx_w, dl_w, dcq_w, tile_of, slot_of = _pack_core(
            cfg, core, src, dst, dis_c, n)
        # xeown: own nodes' features at (tile, slot) columns. xe already
        # carries one dis factor, so xeown @ W1ext = dis*xw = y[n], exactly the
        # self-loop row the segment sum needs (psum is scaled by dis*c later).
        n0 = core * cfg.ndst
        perm = np.full(cfg.tiles * P, -1, dtype=np.int64)
        perm[tile_of * P + slot_of] = np.arange(n0, n0 + cfg.ndst)
        used = perm >= 0
        xeo = np.zeros((B, FEXT, cfg.tiles * P), dtype=np.float32)
        xeo[:, :, used] = xe[:, :, perm[used]]
        m = {"xe": xe, "xeo": xeo, "w1e": w1e, "idxt": idx_w, "dlt": dl_w,
             "dcq": dcq_w, "iot": iota}
        if has_b1:
            disc_w = np.zeros((P, cfg.tiles), dtype=np.float32)
            cc_w = np.zeros((P, cfg.tiles), dtype=np.float32)
            n0 = core * cfg.ndst
            disc_w[slot_of, tile_of] = dis[n0:n0 + cfg.ndst]
            cc_w[slot_of, tile_of] = c[n0:n0 + cfg.ndst]
            m["disc"] = disc_w
            m["cct"] = cc_w
            m["b1b"] = np.tile(b1.astype(np.float32), (P, B))
        in_maps.append(m)
        metas.append((tile_of, slot_of))
    return in_maps, has_b1


def run(inputs, cfg: Cfg = CFG, trace: bool = False):
    node = np.asarray(inputs["node"], dtype=np.float32)
    node_type = np.asarray(inputs["node_type"])
    edge_index = np.asarray(inputs["edge_index"])
    embed = np.asarray(inputs["embed"], dtype=np.float32)
    W1 = np.asarray(inputs["W1"], dtype=np.float32)
    b1 = np.asarray(inputs["b1"], dtype=np.float32)
    W2 = np.asarray(inputs["W2"], dtype=np.float32)
    b2 = np.asarray(inputs["b2"], dtype=np.float32)

    in_maps, has_b1 = _prepare(cfg, node, node_type, edge_index,
                               embed, W1, b1, W2, b2)
    nc = _get_program(cfg, has_b1)
    res = run_bass_kernel_spmd(
        nc, in_maps, core_ids=list(range(cfg.ncores)), trace=trace,
        trace_cores=list(range(cfg.ncores)) if trace else None)

    total = np.zeros((B, H), dtype=np.float64)
    for core in range(cfg.ncores):
        acc = res.results[core]["acc"].astype(np.float64)   # [128, 2*H]
        total += acc.reshape(P, B, H).sum(axis=0)
    out = (total @ W2.astype(np.float64)) / cfg.n + b2.astype(np.float64)
    return out.astype(np.float32), res


def kernel(**inputs) -> np.ndarray:
    out, _ = run(inputs, CFG, trace=False)
    return out



# revision 4
# speedup vs baseline: 1.0385x; 1.0385x over previous
"""Trainium2 Bass kernel for nn_Encoder (2-layer GCN encoder, graph mean readout).

Math restructuring (exact, up to float reordering):
  Layer 1 (GCNConv + ReLU):  x1[n] = relu(dis[n] * (S[n] @ W1ext) + b1),
      S[n] = sum_{e: dst(e)=n} xt[src(e)] + xt[n]        (feature-space sums!)
      xt[m] = dis[m] * xext[m],  xext = [node feats | onehot(type)],
      W1ext = [W1[:116] ; embed @ W1[116:]],  dis = (deg+1)^-1/2.
  Layer 2 + mean over nodes collapses to a per-node scalar:
      out = (1/N) * (sum_n c[n] * x1[n]) @ W2 + b2,
      c[m] = dis[m] * (sum_{e: src(e)=m} dis[dst(e)] + dis[m]).

Because the aggregation is linear, W1 can be applied AFTER the segment sum:
the device gathers raw per-edge feature rows (512B each, fp16, both batches
packed) and segment-sums them in feature space via one-hot matmuls
(lhsT = gathered rows, rhs = one-hot -> psum aggT[feat, dst]); the self-loop
term and the b1/dis row are added as a dense per-tile tile (xgoT) with a
single vector add; then one small matmul per tile/batch applies W1ext.
This removes the dense "phase 1" (x @ W1 for all nodes) entirely: y is never
materialized and each core's HBM traffic is just the edge gather stream.

The gather (SWDGE dma_gather) is descriptor-generation-bound on the Q7 cores
(~8.6us per 1024-idx call on one queue pair).  Calls are spread round-robin
across all 4 SWDGE queues (4 Q7 core pairs work concurrently, measured
2.6ns/idx vs 8.8ns/idx single-queue).

Sharding: destination nodes (and their incoming edges) are split across the
8 cores; the xg feature table is replicated (it is an ExternalInput, no
device cost).  Per-core program is IDENTICAL (SPMD): destination nodes are
bin-packed on host into TILES tiles of 128 psum slots with a fixed
per-src-chunk budget (4 chunks of 128 edges per src quarter), so every
gather call / matmul schedule is a compile-time constant.
"""

import sys, os, types
sys.path.insert(0, "/opt/trn_rl_repo")

# antenv.axon_hooks shim (image's antenv stub lacks it); needed for NTFF trace.
if "antenv.axon_hooks" not in sys.modules:
    _hook = [None]
    _m = types.ModuleType("antenv.axon_hooks")
    _m.set_axon_ntff_profile_hook = lambda h: _hook.__setitem__(0, h)
    _m.get_axon_ntff_profile_hook = lambda: _hook[0]
    sys.modules["antenv.axon_hooks"] = _m
    try:
        import antenv
        antenv.axon_hooks = _m
        from trn_agent_boot.trn_boot import _ntff_profile_via_ctypes
        _m.set_axon_ntff_profile_hook(
            _ntff_profile_via_ctypes("/opt/axon/libaxon_pjrt.so"))
    except Exception:
        pass

import numpy as np
from dataclasses import dataclass

import concourse.bacc as bacc
import concourse.bass as bass
import concourse.mybir as mybir
import concourse.tile as tile
from concourse.bass import AP
from concourse.bass_utils import run_bass_kernel_spmd
from concourse.library_config import mlp

P = 128
H = 128
F_IN = 116
B = 2
ROW = 2 * P               # xg row: [b0 feats+oh | b1 feats+oh], 256 f16
NQ = 4                    # SWDGE queues (4 Q7 core pairs in parallel)


@dataclass(frozen=True)
class Cfg:
    n: int = 100000      # nodes
    ncores: int = 8
    tiles: int = 104     # dst tiles per core (128 slots each)
    chunks: int = 16     # 128-edge chunks per tile (4 per src chunk)
    group: int = 2       # tiles per gather-call group
    nsc: int = 4         # src chunks (int16 gather index reach)

    @property
    def ndst(self):
        return self.n // self.ncores

    @property
    def srcchunk(self):
        return -(-self.n // self.nsc)

    @property
    def npad(self):
        return -(-self.n // 512) * 512

    @property
    def cpt(self):        # chunks per (tile, src chunk)
        assert self.chunks % self.nsc == 0
        return self.chunks // self.nsc

    @property
    def ngroups(self):
        return self.tiles // self.group

    @property
    def call_chunks(self):  # chunks per gather call
        return self.group * self.cpt

    @property
    def call_idx(self):
        return self.call_chunks * P

    @property
    def ncalls(self):
        return self.ngroups * self.nsc

    @property
    def idxcols(self):
        return self.ncalls * (self.call_idx // 16)

    @property
    def nchunks_total(self):
        return self.tiles * self.chunks


CFG = Cfg()

f32 = mybir.dt.float32
f16 = mybir.dt.float16
i16 = mybir.dt.int16


def _build_program(cfg: Cfg):
    nc = bacc.Bacc("TRN2", num_swdge_queues=NQ)
    xg = nc.dram_tensor("xg", [cfg.npad, ROW], f16, kind="ExternalInput")
    w1e = nc.dram_tensor("w1e", [P, H], f16, kind="ExternalInput")
    idxt = nc.dram_tensor("idxt", [P, cfg.idxcols], i16, kind="ExternalInput")
    dlt = nc.dram_tensor("dlt", [P, cfg.nchunks_total], f16,
                         kind="ExternalInput")
    dcq = nc.dram_tensor("dcq", [P, cfg.tiles], f32, kind="ExternalInput")
    iot = nc.dram_tensor("iot", [P, P], f16, kind="ExternalInput")
    xgo = nc.dram_tensor("xgo", [P, cfg.tiles * ROW], f16,
                         kind="ExternalInput")
    accd = nc.dram_tensor("acc", [P, ROW], f32, kind="ExternalOutput")

    with tile.TileContext(nc) as tc:
        nc.gpsimd.load_library(mlp)
        with (
            tc.tile_pool(name="const", bufs=1) as cpool,
            tc.tile_pool(name="gat", bufs=12) as gpool,
            tc.tile_pool(name="oh", bufs=3) as ohpool,
            tc.tile_pool(name="agg", bufs=4) as apool,
            tc.tile_pool(name="x1c", bufs=4) as xpool,
            tc.tile_pool(name="psa", bufs=4, space="PSUM") as psa,
            tc.tile_pool(name="pso", bufs=4, space="PSUM") as pso,
        ):
            # constants / preloads: idx table first (gates the first gather),
            # split so early gathers start after the first slice lands.
            idx_sb = cpool.tile([P, cfg.idxcols], i16, tag="idx")
            nsplit = 8
            step = cfg.idxcols // nsplit
            for i in range(nsplit):
                nc.sync.dma_start(idx_sb[:, i * step:(i + 1) * step],
                                  idxt[:, i * step:(i + 1) * step])
            w1_sb = cpool.tile([P, H], f16, tag="w1")
            nc.sync.dma_start(w1_sb[:], w1e[:])
            iota_sb = cpool.tile([P, P], f16, tag="iota")
            nc.sync.dma_start(iota_sb[:], iot[:])
            dl_sb = cpool.tile([P, cfg.nchunks_total], f16, tag="dl")
            nc.sync.dma_start(dl_sb[:], dlt[:])
            dcq_sb = cpool.tile([P, cfg.tiles], f32, tag="dcq")
            nc.sync.dma_start(dcq_sb[:], dcq[:])
            xgo_sb = cpool.tile([P, cfg.tiles * ROW], f16, tag="xgo")
            nc.sync.dma_start(xgo_sb[:], xgo[:])
            acc_sb = cpool.tile([P, ROW], f32, tag="acc")
            nc.vector.memset(acc_sb[:], 0)

            ic_per_call = cfg.call_idx // 16
            iota_ap = iota_sb[:, :]
            iota_3d = AP(iota_ap.tensor, iota_ap.offset,
                         [iota_ap.ap[0], (0, cfg.chunks), iota_ap.ap[1]])

            for g in range(cfg.ngroups):
                gts = []
                for s in range(cfg.nsc):
                    call = g * cfg.nsc + s
                    ic0 = call * ic_per_call
                    gt = gpool.tile([P, cfg.call_chunks, ROW], f16, tag="gt")
                    r0 = s * cfg.srcchunk
                    nc.gpsimd.dma_gather(
                        gt[:], xg[r0:r0 + cfg.srcchunk, :],
                        idx_sb[:, ic0:ic0 + ic_per_call],
                        cfg.call_idx, cfg.call_idx, ROW,
                        queue_num=s % NQ, single_packet=False)
                    gts.append(gt)
                for ti in range(cfg.group):
                    t = g * cfg.group + ti
                    # bulk one-hot build: oh_all[:, k, :] for the 16 chunks
                    # of tile t (k = s*cpt + j, host lays dlt out to match)
                    oh_all = ohpool.tile([P, cfg.chunks, P], f16, tag="oh")
                    dl_slice = dl_sb[:, t * cfg.chunks:(t + 1) * cfg.chunks]
                    nc.vector.tensor_tensor(
                        out=oh_all[:],
                        in0=dl_slice.to_broadcast([P, cfg.chunks, P]),
                        in1=iota_3d,
                        op=mybir.AluOpType.is_equal)
                    ps = psa.tile([P, ROW], f32, tag="ps", name=f"ps{t}")
                    start_mm = None
                    off = ti * cfg.cpt
                    for s in range(cfg.nsc):
                        for j in range(cfg.cpt):
                            k = s * cfg.cpt + j
                            last = (k == cfg.chunks - 1)
                            # aggT[feat, dst] += gathered_chunk.T @ onehot
                            # exactly one start=True matmul per psum tile
                            # (PSUM zero-regions are 2KB: start marks the
                            # whole region pending-zero -> unique + first)
                            mm0 = nc.tensor.matmul(
                                ps[:, 0:P],
                                lhsT=gts[s][:, off + j, 0:P],
                                rhs=oh_all[:, k, :],
                                start=(k == 0), stop=last)
                            mm1 = nc.tensor.matmul(
                                ps[:, P:ROW],
                                lhsT=gts[s][:, off + j, P:ROW],
                                rhs=oh_all[:, k, :],
                                start=False, stop=last)
                            if k == 0:
                                start_mm = mm0
                                bass._add_dep_helper(
                                    mm1.ins, start_mm.ins, sync=False,
                                    reason="half1 after psum start")
                            else:
                                for mm in (mm0, mm1):
                                    bass._add_dep_helper(
                                        mm.ins, start_mm.ins, sync=False,
                                        reason="accum after psum start")
                    # aggT += self-loop features (+ b1/dis row): dense add
                    agg_sb = apool.tile([P, ROW], f16, tag="agg")
                    nc.vector.tensor_tensor(
                        out=agg_sb[:], in0=ps[:],
                        in1=xgo_sb[:, t * ROW:(t + 1) * ROW],
                        op=mybir.AluOpType.add)
                    # out1[dst, h] = aggT.T @ W1ext  (per batch half)
                    po = pso.tile([P, ROW], f32, tag="po", name=f"po{t}")
                    wm0 = nc.tensor.matmul(
                        po[:, 0:P], lhsT=agg_sb[:, 0:P], rhs=w1_sb[:],
                        start=True, stop=True)
                    wm1 = nc.tensor.matmul(
                        po[:, P:ROW], lhsT=agg_sb[:, P:ROW], rhs=w1_sb[:],
                        start=False, stop=True)
                    bass._add_dep_helper(
                        wm1.ins, wm0.ins, sync=False,
                        reason="half1 after psum start")
                    # x1c = relu(out1 * dis*c)  (dis,c > 0 so scale commutes)
                    x1c = xpool.tile([P, ROW], f32, tag="x1c")
                    nc.scalar.activation(
                        out=x1c[:], in_=po[:],
                        func=mybir.ActivationFunctionType.Relu,
                        bias=0.0, scale=dcq_sb[:, t:t + 1])
                    nc.vector.tensor_tensor(
                        out=acc_sb[:], in0=acc_sb[:], in1=x1c[:],
                        op=mybir.AluOpType.add)

            nc.sync.dma_start(accd[:], acc_sb[:])

    nc.compile()
    return nc


_PROG_CACHE = {}


def _get_program(cfg: Cfg):
    if cfg not in _PROG_CACHE:
        _PROG_CACHE[cfg] = _build_program(cfg)
    return _PROG_CACHE[cfg]


def _pack_core(cfg: Cfg, core, src, dst):
    """Bin-pack this core's dst nodes into tiles; build gather/dstloc data.

    Returns (idx_w [128, idxcols] i16, dl2_w [128, nchunks] f16 in per-tile
    chunk order, tile_of, slot_of)."""
    n0 = core * cfg.ndst
    sel = (dst >= n0) & (dst < n0 + cfg.ndst)
    es = src[sel]
    ed = dst[sel]
    dl = ed - n0                       # local dst id
    sc = es // cfg.srcchunk            # src chunk of each edge

    cnt = np.bincount(dl * cfg.nsc + sc, minlength=cfg.ndst * cfg.nsc)
    cnt = cnt.reshape(cfg.ndst, cfg.nsc)

    caps = np.full((cfg.tiles, cfg.nsc), cfg.cpt * P, dtype=np.int64)
    for s in range(cfg.nsc):
        assert cnt[:, s].sum() <= caps[:, s].sum(), \
            f"core {core}: src chunk {s} demand exceeds capacity"

    order = np.argsort(-cnt.sum(1), kind="stable")
    slots_used = np.zeros(cfg.tiles, dtype=np.int64)
    tile_of = np.full(cfg.ndst, -1, dtype=np.int64)
    slot_of = np.full(cfg.ndst, -1, dtype=np.int64)
    for nloc in order:
        need = cnt[nloc]
        ok = (caps >= need).all(axis=1) & (slots_used < P)
        if not ok.any():
            raise RuntimeError(f"core {core}: bin packing failed for node {nloc}")
        score = caps.sum(axis=1) + (P - slots_used)
        score[~ok] = -1
        t = int(np.argmax(score))
        tile_of[nloc] = t
        slot_of[nloc] = slots_used[t]
        slots_used[t] += 1
        caps[t] -= need

    # edge stream positions
    et = tile_of[dl]
    eslot = slot_of[dl]
    o = np.lexsort((sc, et))
    et_s, sc_s, slot_s, src_s = et[o], sc[o], eslot[o], es[o]
    ks = et_s * cfg.nsc + sc_s
    counts = np.bincount(ks, minlength=cfg.tiles * cfg.nsc)
    gbase = np.concatenate([[0], np.cumsum(counts)[:-1]])
    rank = np.arange(len(ks)) - gbase[ks]

    # stream base for (t, s): call (t//group, s), offset (t%group)*cpt chunks
    tt = np.arange(cfg.tiles)
    callno = (tt // cfg.group)[:, None] * cfg.nsc + np.arange(cfg.nsc)[None, :]
    pbase = callno * cfg.call_idx + (tt % cfg.group)[:, None] * cfg.cpt * P
    assert (counts.reshape(cfg.tiles, cfg.nsc) <= cfg.cpt * P).all()

    total = cfg.ncalls * cfg.call_idx
    idx_flat = np.zeros(total, dtype=np.int16)
    dl_flat = np.full(total, 255.0, dtype=np.float16)
    pos = pbase[et_s, sc_s] + rank
    idx_flat[pos] = (src_s - sc_s * cfg.srcchunk).astype(np.int16)
    dl_flat[pos] = slot_s.astype(np.float16)

    ci = cfg.call_idx
    idx_w = idx_flat.reshape(cfg.ncalls, ci // 16, 16).transpose(2, 0, 1)
    idx_w = np.tile(idx_w.reshape(16, -1), (8, 1))           # [128, idxcols]

    # dl in per-tile chunk order: column t*chunks + s*cpt + j holds the slot
    # ids of the chunk at stream call (t//group, s), chunk (t%group)*cpt + j.
    dl_st = dl_flat.reshape(cfg.ncalls, cfg.call_chunks, P)  # [call, chunk, P]
    dl2 = np.empty((cfg.tiles, cfg.chunks, P), dtype=np.float16)
    for ti in range(cfg.group):
        for s in range(cfg.nsc):
            # tiles with t%group==ti, their cpt chunks from call (g, s)
            dl2[ti::cfg.group, s * cfg.cpt:(s + 1) * cfg.cpt, :] = \
                dl_st[s::cfg.nsc, ti * cfg.cpt:(ti + 1) * cfg.cpt, :][
                    :cfg.ngroups]
    dl2_w = dl2.reshape(cfg.tiles * cfg.chunks, P).T.copy()  # [128, nchunks]
    return idx_w, dl2_w, tile_of, slot_of


def _prepare(cfg: Cfg, node, node_type, edge_index, embed, W1, b1):
    n = cfg.n
    src = edge_index[0].astype(np.int64)
    dst = edge_index[1].astype(np.int64)
    deg = (np.bincount(dst, minlength=n) + 1).astype(np.float64)
    dis64 = 1.0 / np.sqrt(deg)
    dis = dis64.astype(np.float32)
    s_arr = np.bincount(src, weights=dis64[dst], minlength=n)
    c = (dis64 * (s_arr + dis64)).astype(np.float32)
    dis_c = (dis64 * c.astype(np.float64)).astype(np.float32)

    T8 = embed.astype(np.float64) @ W1[F_IN:, :].astype(np.float64)
    w1e = np.zeros((P, H), dtype=np.float16)
    w1e[:F_IN] = W1[:F_IN]
    w1e[F_IN:F_IN + 8] = T8.astype(np.float16)
    w1e[F_IN + 8] = b1                      # b1 feature row (see xgo)

    # xg rows: [dis*node_b0 | dis*onehot | 0 pad] x2 halves
    xg = np.zeros((cfg.npad, ROW), dtype=np.float16)
    for b in range(B):
        xg[:n, b * P:b * P + F_IN] = node[b] * dis[:, None]
    oh_col = F_IN + node_type.astype(np.int64)          # one-hot position
    rows = np.arange(n)
    for b in range(B):
        xg[rows, b * P + oh_col] = dis
    # (row F_IN+8 stays 0 in xg: the b1 feature enters via xgo only)

    iota = np.tile(np.arange(P, dtype=np.float16), (P, 1))

    in_maps = []
    for core in range(cfg.ncores):
        idx_w, dl2_w, tile_of, slot_of = _pack_core(cfg, core, src, dst)
        n0 = core * cfg.ndst
        # xgo: per (tile, slot) the node's own xg row (self-loop term), with
        # the b1 feature slot set to 1/dis so out1 picks up b1/dis.
        nodes = np.arange(n0, n0 + cfg.ndst)
        xrows = xg[nodes].astype(np.float32)
        inv_dis = (1.0 / dis[nodes]).astype(np.float32)
        xrows[:, F_IN + 8] = inv_dis
        xrows[:, P + F_IN + 8] = inv_dis
        xgo = np.zeros((P, cfg.tiles * ROW), dtype=np.float16)
        # columns: t*ROW + half*P + slot
        for hf in range(2):
            colidx = tile_of * ROW + hf * P + slot_of
            xgo[:, colidx] = xrows[:, hf * P:(hf + 1) * P].T.astype(np.float16)
        dcq_w = np.zeros((P, cfg.tiles), dtype=np.float32)
        dcq_w[slot_of, tile_of] = dis_c[n0:n0 + cfg.ndst]
        m = {"xg": xg, "w1e": w1e, "idxt": idx_w, "dlt": dl2_w,
             "dcq": dcq_w, "iot": iota, "xgo": xgo}
        in_maps.append(m)
    return in_maps


def run(inputs, cfg: Cfg = CFG, trace: bool = False):
    node = np.asarray(inputs["node"], dtype=np.float32)
    node_type = np.asarray(inputs["node_type"])
    edge_index = np.asarray(inputs["edge_index"])
    embed = np.asarray(inputs["embed"], dtype=np.float32)
    W1 = np.asarray(inputs["W1"], dtype=np.float32)
    b1 = np.asarray(inputs["b1"], dtype=np.float32)
    W2 = np.asarray(inputs["W2"], dtype=np.float32)
    b2 = np.asarray(inputs["b2"], dtype=np.float32)

    in_maps = _prepare(cfg, node, node_type, edge_index, embed, W1, b1)
    nc = _get_program(cfg)
    res = run_bass_kernel_spmd(
        nc, in_maps, core_ids=list(range(cfg.ncores)), trace=trace,
        trace_cores=list(range(cfg.ncores)) if trace else None)

    total = np.zeros((B, H), dtype=np.float64)
    for core in range(cfg.ncores):
        acc = res.results[core]["acc"].astype(np.float64)   # [128, 2*H]
        total += acc.reshape(P, B, H).sum(axis=0)
    out = (total @ W2.astype(np.float64)) / cfg.n + b2.astype(np.float64)
    return out.astype(np.float32), res


def kernel(**inputs) -> np.ndarray:
    out, _ = run(inputs, CFG, trace=False)
    return out


# revision 6
# speedup vs baseline: 1.0822x; 1.0421x over previous
"""Trainium2 Bass kernel for nn_Encoder (2-layer GCN encoder, graph mean readout).

Math restructuring (exact, up to float reordering):
  Layer 1 (GCNConv + ReLU):  x1[n] = relu(dis[n] * (S[n] @ W1ext) + b1),
      S[n] = sum_{e: dst(e)=n} xt[src(e)] + xt[n]        (feature-space sums!)
      xt[m] = dis[m] * xext[m],  xext = [node feats | onehot(type)],
      W1ext = [W1[:116] ; embed @ W1[116:]],  dis = (deg+1)^-1/2.
  Layer 2 + mean over nodes collapses to a per-node scalar:
      out = (1/N) * (sum_n c[n] * x1[n]) @ W2 + b2,
      c[m] = dis[m] * (sum_{e: src(e)=m} dis[dst(e)] + dis[m]).

Because the aggregation is linear, W1 can be applied AFTER the segment sum:
the device gathers raw per-edge feature rows (512B each, fp16, both batches
packed) and segment-sums them in feature space via one-hot matmuls
(lhsT = gathered rows, rhs = one-hot -> psum aggT[feat, dst]); the self-loop
term and the b1/dis row are added as a dense per-tile tile (xgoT) with a
single vector add; then one small matmul per tile/batch applies W1ext.
This removes the dense "phase 1" (x @ W1 for all nodes) entirely: y is never
materialized and each core's HBM traffic is just the edge gather stream.

The gather (SWDGE dma_gather) is descriptor-generation-bound on the Q7 cores
(~8.6us per 1024-idx call on one queue pair).  Calls are spread round-robin
across all 4 SWDGE queues (4 Q7 core pairs work concurrently, measured
2.6ns/idx vs 8.8ns/idx single-queue).

Sharding: destination nodes (and their incoming edges) are split across the
8 cores; the xg feature table is replicated (it is an ExternalInput, no
device cost).  Per-core program is IDENTICAL (SPMD): destination nodes are
bin-packed on host into TILES tiles of 128 psum slots with a fixed
per-src-chunk budget (4 chunks of 128 edges per src quarter), so every
gather call / matmul schedule is a compile-time constant.
"""

import sys, os, types
sys.path.insert(0, "/opt/trn_rl_repo")

# antenv.axon_hooks shim (image's antenv stub lacks it); needed for NTFF trace.
if "antenv.axon_hooks" not in sys.modules:
    _hook = [None]
    _m = types.ModuleType("antenv.axon_hooks")
    _m.set_axon_ntff_profile_hook = lambda h: _hook.__setitem__(0, h)
    _m.get_axon_ntff_profile_hook = lambda: _hook[0]
    sys.modules["antenv.axon_hooks"] = _m
    try:
        import antenv
        antenv.axon_hooks = _m
        from trn_agent_boot.trn_boot import _ntff_profile_via_ctypes
        _m.set_axon_ntff_profile_hook(
            _ntff_profile_via_ctypes("/opt/axon/libaxon_pjrt.so"))
    except Exception:
        pass

import numpy as np
from dataclasses import dataclass

import concourse.bacc as bacc
import concourse.bass as bass
import concourse.mybir as mybir
import concourse.tile as tile
from concourse.bass import AP
from concourse.bass_utils import run_bass_kernel_spmd
from concourse.library_config import mlp

P = 128
H = 128
F_IN = 116
B = 2
ROW = 2 * P               # xg row: [b0 feats+oh | b1 feats+oh], 256 f16
NQ = 4                    # SWDGE queues (4 Q7 core pairs in parallel)


@dataclass(frozen=True)
class Cfg:
    n: int = 100000      # nodes
    ncores: int = 8
    tiles: int = 100     # dst tiles per core (128 slots each)
    chunks: int = 16     # 128-edge chunks per tile (4 per src chunk)
    group: int = 2       # tiles per gather-call group
    nsc: int = 4         # src chunks (int16 gather index reach)

    @property
    def ndst(self):
        return self.n // self.ncores

    @property
    def srcchunk(self):
        return -(-self.n // self.nsc)

    @property
    def npad(self):
        return -(-self.n // 512) * 512

    @property
    def cpt(self):        # chunks per (tile, src chunk)
        assert self.chunks % self.nsc == 0
        return self.chunks // self.nsc

    @property
    def ngroups(self):
        return self.tiles // self.group

    @property
    def call_chunks(self):  # chunks per gather call
        return self.group * self.cpt

    @property
    def call_idx(self):
        return self.call_chunks * P

    @property
    def ncalls(self):
        return self.ngroups * self.nsc

    @property
    def idxcols(self):
        return self.ncalls * (self.call_idx // 16)

    @property
    def nchunks_total(self):
        return self.tiles * self.chunks


CFG = Cfg()

f32 = mybir.dt.float32
f16 = mybir.dt.float16
i16 = mybir.dt.int16


def _build_program(cfg: Cfg):
    nc = bacc.Bacc("TRN2", num_swdge_queues=NQ)
    xg = nc.dram_tensor("xg", [cfg.npad, ROW], f16, kind="ExternalInput")
    w1e = nc.dram_tensor("w1e", [P, H], f16, kind="ExternalInput")
    idxt = nc.dram_tensor("idxt", [P, cfg.idxcols], i16, kind="ExternalInput")
    dlt = nc.dram_tensor("dlt", [P, cfg.nchunks_total], f16,
                         kind="ExternalInput")
    dcq = nc.dram_tensor("dcq", [P, cfg.tiles], f32, kind="ExternalInput")
    iot = nc.dram_tensor("iot", [P, P], f16, kind="ExternalInput")
    xgo = nc.dram_tensor("xgo", [P, cfg.tiles * ROW], f16,
                         kind="ExternalInput")
    accd = nc.dram_tensor("acc", [P, ROW], f32, kind="ExternalOutput")

    with tile.TileContext(nc) as tc:
        nc.gpsimd.load_library(mlp)
        with (
            tc.tile_pool(name="const", bufs=1) as cpool,
            tc.tile_pool(name="gat", bufs=12) as gpool,
            tc.tile_pool(name="oh", bufs=3) as ohpool,
            tc.tile_pool(name="agg", bufs=4) as apool,
            tc.tile_pool(name="x1c", bufs=4) as xpool,
            tc.tile_pool(name="psa", bufs=4, space="PSUM") as psa,
            tc.tile_pool(name="pso", bufs=4, space="PSUM") as pso,
        ):
            # constants / preloads: idx table first (gates the first gather),
            # split so early gathers start after the first slice lands.
            idx_sb = cpool.tile([P, cfg.idxcols], i16, tag="idx")
            nsplit = 8
            step = cfg.idxcols // nsplit
            for i in range(nsplit):
                nc.sync.dma_start(idx_sb[:, i * step:(i + 1) * step],
                                  idxt[:, i * step:(i + 1) * step])
            w1_sb = cpool.tile([P, H], f16, tag="w1")
            nc.sync.dma_start(w1_sb[:], w1e[:])
            iota_sb = cpool.tile([P, P], f16, tag="iota")
            nc.sync.dma_start(iota_sb[:], iot[:])
            dl_sb = cpool.tile([P, cfg.nchunks_total], f16, tag="dl")
            nc.sync.dma_start(dl_sb[:], dlt[:])
            dcq_sb = cpool.tile([P, cfg.tiles], f32, tag="dcq")
            nc.sync.dma_start(dcq_sb[:], dcq[:])
            xgo_sb = cpool.tile([P, cfg.tiles * ROW], f16, tag="xgo")
            nc.sync.dma_start(xgo_sb[:], xgo[:])
            acc_sb = cpool.tile([P, ROW], f32, tag="acc")
            nc.vector.memset(acc_sb[:], 0)

            ic_per_call = cfg.call_idx // 16
            iota_ap = iota_sb[:, :]
            iota_3d = AP(iota_ap.tensor, iota_ap.offset,
                         [iota_ap.ap[0], (0, cfg.chunks), iota_ap.ap[1]])

            for g in range(cfg.ngroups):
                gts = []
                for s in range(cfg.nsc):
                    call = g * cfg.nsc + s
                    ic0 = call * ic_per_call
                    gt = gpool.tile([P, cfg.call_chunks, ROW], f16, tag="gt")
                    r0 = s * cfg.srcchunk
                    nc.gpsimd.dma_gather(
                        gt[:], xg[r0:r0 + cfg.srcchunk, :],
                        idx_sb[:, ic0:ic0 + ic_per_call],
                        cfg.call_idx, cfg.call_idx, ROW,
                        queue_num=s % NQ, single_packet=False)
                    gts.append(gt)
                for ti in range(cfg.group):
                    t = g * cfg.group + ti
                    # bulk one-hot build: oh_all[:, k, :] for the 16 chunks
                    # of tile t (k = s*cpt + j, host lays dlt out to match)
                    oh_all = ohpool.tile([P, cfg.chunks, P], f16, tag="oh")
                    dl_slice = dl_sb[:, t * cfg.chunks:(t + 1) * cfg.chunks]
                    nc.vector.tensor_tensor(
                        out=oh_all[:],
                        in0=dl_slice.to_broadcast([P, cfg.chunks, P]),
                        in1=iota_3d,
                        op=mybir.AluOpType.is_equal)
                    ps = psa.tile([P, ROW], f32, tag="ps", name=f"ps{t}")
                    start_mm = None
                    off = ti * cfg.cpt
                    for s in range(cfg.nsc):
                        for j in range(cfg.cpt):
                            k = s * cfg.cpt + j
                            last = (k == cfg.chunks - 1)
                            # aggT[feat, dst] += gathered_chunk.T @ onehot
                            # exactly one start=True matmul per psum tile
                            # (PSUM zero-regions are 2KB: start marks the
                            # whole region pending-zero -> unique + first)
                            mm0 = nc.tensor.matmul(
                                ps[:, 0:P],
                                lhsT=gts[s][:, off + j, 0:P],
                                rhs=oh_all[:, k, :],
                                start=(k == 0), stop=last)
                            mm1 = nc.tensor.matmul(
                                ps[:, P:ROW],
                                lhsT=gts[s][:, off + j, P:ROW],
                                rhs=oh_all[:, k, :],
                                start=False, stop=last)
                            if k == 0:
                                start_mm = mm0
                                bass._add_dep_helper(
                                    mm1.ins, start_mm.ins, sync=False,
                                    reason="half1 after psum start")
                            else:
                                for mm in (mm0, mm1):
                                    bass._add_dep_helper(
                                        mm.ins, start_mm.ins, sync=False,
                                        reason="accum after psum start")
                    # aggT += self-loop features (+ b1/dis row): dense add
                    agg_sb = apool.tile([P, ROW], f16, tag="agg")
                    nc.vector.tensor_tensor(
                        out=agg_sb[:], in0=ps[:],
                        in1=xgo_sb[:, t * ROW:(t + 1) * ROW],
                        op=mybir.AluOpType.add)
                    # out1[dst, h] = aggT.T @ W1ext  (per batch half)
                    po = pso.tile([P, ROW], f32, tag="po", name=f"po{t}")
                    wm0 = nc.tensor.matmul(
                        po[:, 0:P], lhsT=agg_sb[:, 0:P], rhs=w1_sb[:],
                        start=True, stop=True)
                    wm1 = nc.tensor.matmul(
                        po[:, P:ROW], lhsT=agg_sb[:, P:ROW], rhs=w1_sb[:],
                        start=False, stop=True)
                    bass._add_dep_helper(
                        wm1.ins, wm0.ins, sync=False,
                        reason="half1 after psum start")
                    # x1c = relu(out1 * dis*c)  (dis,c > 0 so scale commutes)
                    x1c = xpool.tile([P, ROW], f32, tag="x1c")
                    nc.scalar.activation(
                        out=x1c[:], in_=po[:],
                        func=mybir.ActivationFunctionType.Relu,
                        bias=0.0, scale=dcq_sb[:, t:t + 1])
                    nc.vector.tensor_tensor(
                        out=acc_sb[:], in0=acc_sb[:], in1=x1c[:],
                        op=mybir.AluOpType.add)

            nc.sync.dma_start(accd[:], acc_sb[:])

    nc.compile()
    return nc


_PROG_CACHE = {}


def _get_program(cfg: Cfg):
    if cfg not in _PROG_CACHE:
        _PROG_CACHE[cfg] = _build_program(cfg)
    return _PROG_CACHE[cfg]


def _pack_core(cfg: Cfg, core, src, dst):
    """Bin-pack this core's dst nodes into tiles; build gather/dstloc data.

    Returns (idx_w [128, idxcols] i16, dl2_w [128, nchunks] f16 in per-tile
    chunk order, tile_of, slot_of)."""
    n0 = core * cfg.ndst
    sel = (dst >= n0) & (dst < n0 + cfg.ndst)
    es = src[sel]
    ed = dst[sel]
    dl = ed - n0                       # local dst id
    sc = es // cfg.srcchunk            # src chunk of each edge

    cnt = np.bincount(dl * cfg.nsc + sc, minlength=cfg.ndst * cfg.nsc)
    cnt = cnt.reshape(cfg.ndst, cfg.nsc)

    caps = np.full((cfg.tiles, cfg.nsc), cfg.cpt * P, dtype=np.int64)
    for s in range(cfg.nsc):
        assert cnt[:, s].sum() <= caps[:, s].sum(), \
            f"core {core}: src chunk {s} demand exceeds capacity"

    # order by worst-dim demand; place to maximize the min remaining
    # per-src-chunk capacity (multi-dim balance packs ~98% full)
    order = np.argsort(-cnt.max(1), kind="stable")
    slots_used = np.zeros(cfg.tiles, dtype=np.int64)
    tile_of = np.full(cfg.ndst, -1, dtype=np.int64)
    slot_of = np.full(cfg.ndst, -1, dtype=np.int64)
    for nloc in order:
        need = cnt[nloc]
        ok = (caps >= need).all(axis=1) & (slots_used < P)
        if not ok.any():
            raise RuntimeError(f"core {core}: bin packing failed for node {nloc}")
        score = (caps - need).min(axis=1) * 1000 + (P - slots_used)
        score = np.where(ok, score, -1)
        t = int(np.argmax(score))
        tile_of[nloc] = t
        slot_of[nloc] = slots_used[t]
        slots_used[t] += 1
        caps[t] -= need

    # edge stream positions
    et = tile_of[dl]
    eslot = slot_of[dl]
    o = np.lexsort((sc, et))
    et_s, sc_s, slot_s, src_s = et[o], sc[o], eslot[o], es[o]
    ks = et_s * cfg.nsc + sc_s
    counts = np.bincount(ks, minlength=cfg.tiles * cfg.nsc)
    gbase = np.concatenate([[0], np.cumsum(counts)[:-1]])
    rank = np.arange(len(ks)) - gbase[ks]

    # stream base for (t, s): call (t//group, s), offset (t%group)*cpt chunks
    tt = np.arange(cfg.tiles)
    callno = (tt // cfg.group)[:, None] * cfg.nsc + np.arange(cfg.nsc)[None, :]
    pbase = callno * cfg.call_idx + (tt % cfg.group)[:, None] * cfg.cpt * P
    assert (counts.reshape(cfg.tiles, cfg.nsc) <= cfg.cpt * P).all()

    total = cfg.ncalls * cfg.call_idx
    idx_flat = np.zeros(total, dtype=np.int16)
    dl_flat = np.full(total, 255.0, dtype=np.float16)
    pos = pbase[et_s, sc_s] + rank
    idx_flat[pos] = (src_s - sc_s * cfg.srcchunk).astype(np.int16)
    dl_flat[pos] = slot_s.astype(np.float16)

    ci = cfg.call_idx
    idx_w = idx_flat.reshape(cfg.ncalls, ci // 16, 16).transpose(2, 0, 1)
    idx_w = np.tile(idx_w.reshape(16, -1), (8, 1))           # [128, idxcols]

    # dl in per-tile chunk order: column t*chunks + s*cpt + j holds the slot
    # ids of the chunk at stream call (t//group, s), chunk (t%group)*cpt + j.
    dl_st = dl_flat.reshape(cfg.ncalls, cfg.call_chunks, P)  # [call, chunk, P]
    dl2 = np.empty((cfg.tiles, cfg.chunks, P), dtype=np.float16)
    for ti in range(cfg.group):
        for s in range(cfg.nsc):
            # tiles with t%group==ti, their cpt chunks from call (g, s)
            dl2[ti::cfg.group, s * cfg.cpt:(s + 1) * cfg.cpt, :] = \
                dl_st[s::cfg.nsc, ti * cfg.cpt:(ti + 1) * cfg.cpt, :][
                    :cfg.ngroups]
    dl2_w = dl2.reshape(cfg.tiles * cfg.chunks, P).T.copy()  # [128, nchunks]
    return idx_w, dl2_w, tile_of, slot_of


def _prepare(cfg: Cfg, node, node_type, edge_index, embed, W1, b1):
    n = cfg.n
    src = edge_index[0].astype(np.int64)
    dst = edge_index[1].astype(np.int64)
    deg = (np.bincount(dst, minlength=n) + 1).astype(np.float64)
    dis64 = 1.0 / np.sqrt(deg)
    dis = dis64.astype(np.float32)
    s_arr = np.bincount(src, weights=dis64[dst], minlength=n)
    c = (dis64 * (s_arr + dis64)).astype(np.float32)
    dis_c = (dis64 * c.astype(np.float64)).astype(np.float32)

    T8 = embed.astype(np.float64) @ W1[F_IN:, :].astype(np.float64)
    w1e = np.zeros((P, H), dtype=np.float16)
    w1e[:F_IN] = W1[:F_IN]
    w1e[F_IN:F_IN + 8] = T8.astype(np.float16)
    w1e[F_IN + 8] = b1                      # b1 feature row (see xgo)

    # xg rows: [dis*node_b0 | dis*onehot | 0 pad] x2 halves
    xg = np.zeros((cfg.npad, ROW), dtype=np.float16)
    for b in range(B):
        xg[:n, b * P:b * P + F_IN] = node[b] * dis[:, None]
    oh_col = F_IN + node_type.astype(np.int64)          # one-hot position
    rows = np.arange(n)
    for b in range(B):
        xg[rows, b * P + oh_col] = dis
    # (row F_IN+8 stays 0 in xg: the b1 feature enters via xgo only)

    iota = np.tile(np.arange(P, dtype=np.float16), (P, 1))

    in_maps = []
    for core in range(cfg.ncores):
        idx_w, dl2_w, tile_of, slot_of = _pack_core(cfg, core, src, dst)
        n0 = core * cfg.ndst
        # xgo: per (tile, slot) the node's own xg row (self-loop term), with
        # the b1 feature slot set to 1/dis so out1 picks up b1/dis.
        nodes = np.arange(n0, n0 + cfg.ndst)
        xrows = xg[nodes].astype(np.float32)
        inv_dis = (1.0 / dis[nodes]).astype(np.float32)
        xrows[:, F_IN + 8] = inv_dis
        xrows[:, P + F_IN + 8] = inv_dis
        xgo = np.zeros((P, cfg.tiles * ROW), dtype=np.float16)
        # columns: t*ROW + half*P + slot
        for hf in range(2):
            colidx = tile_of * ROW + hf * P + slot_of
            xgo[:, colidx] = xrows[:, hf * P:(hf + 1) * P].T.astype(np.float16)
        dcq_w = np.zeros((P, cfg.tiles), dtype=np.float32)
        dcq_w[slot_of, tile_of] = dis_c[n0:n0 + cfg.ndst]
        m = {"xg": xg, "w1e": w1e, "idxt": idx_w, "dlt": dl2_w,
             "dcq": dcq_w, "iot": iota, "xgo": xgo}
        in_maps.append(m)
    return in_maps


def run(inputs, cfg: Cfg = CFG, trace: bool = False):
    node = np.asarray(inputs["node"], dtype=np.float32)
    node_type = np.asarray(inputs["node_type"])
    edge_index = np.asarray(inputs["edge_index"])
    embed = np.asarray(inputs["embed"], dtype=np.float32)
    W1 = np.asarray(inputs["W1"], dtype=np.float32)
    b1 = np.asarray(inputs["b1"], dtype=np.float32)
    W2 = np.asarray(inputs["W2"], dtype=np.float32)
    b2 = np.asarray(inputs["b2"], dtype=np.float32)

    in_maps = _prepare(cfg, node, node_type, edge_index, embed, W1, b1)
    nc = _get_program(cfg)
    res = run_bass_kernel_spmd(
        nc, in_maps, core_ids=list(range(cfg.ncores)), trace=trace,
        trace_cores=list(range(cfg.ncores)) if trace else None)

    total = np.zeros((B, H), dtype=np.float64)
    for core in range(cfg.ncores):
        acc = res.results[core]["acc"].astype(np.float64)   # [128, 2*H]
        total += acc.reshape(P, B, H).sum(axis=0)
    out = (total @ W2.astype(np.float64)) / cfg.n + b2.astype(np.float64)
    return out.astype(np.float32), res


def kernel(**inputs) -> np.ndarray:
    out, _ = run(inputs, CFG, trace=False)
    return out


# revision 8
# speedup vs baseline: 1.1026x; 1.0189x over previous
"""Trainium2 Bass kernel for nn_Encoder (2-layer GCN encoder, graph mean readout).

Math restructuring (exact, up to float reordering):
  Layer 1 (GCNConv + ReLU):  x1[n] = relu(dis[n] * (S[n] @ W1ext) + b1),
      S[n] = sum_{e: dst(e)=n} xt[src(e)] + xt[n]        (feature-space sums!)
      xt[m] = dis[m] * xext[m],  xext = [node feats | onehot(type)],
      W1ext = [W1[:116] ; embed @ W1[116:]],  dis = (deg+1)^-1/2.
  Layer 2 + mean over nodes collapses to a per-node scalar:
      out = (1/N) * (sum_n c[n] * x1[n]) @ W2 + b2,
      c[m] = dis[m] * (sum_{e: src(e)=m} dis[dst(e)] + dis[m]).

Because the aggregation is linear, W1 can be applied AFTER the segment sum:
the device gathers raw per-edge feature rows (512B each, fp16, both batches
packed) and segment-sums them in feature space via one-hot matmuls
(lhsT = gathered rows, rhs = one-hot -> psum aggT[feat, dst]); the self-loop
term and the b1/dis row are added as a dense per-tile tile (xgoT) with a
single vector add; then one small matmul per tile/batch applies W1ext.
This removes the dense "phase 1" (x @ W1 for all nodes) entirely: y is never
materialized and each core's HBM traffic is just the edge gather stream.

The gather (SWDGE dma_gather) is descriptor-generation-bound on the Q7 cores
(~8.6us per 1024-idx call on one queue pair).  Calls are spread round-robin
across all 4 SWDGE queues (4 Q7 core pairs work concurrently, measured
2.6ns/idx vs 8.8ns/idx single-queue).

Sharding: destination nodes (and their incoming edges) are split across the
8 cores; the xg feature table is replicated (it is an ExternalInput, no
device cost).  Per-core program is IDENTICAL (SPMD): destination nodes are
bin-packed on host into 100 tiles of 128 psum slots with a fixed
per-src-chunk budget (4 chunks of 128 edges per src quarter), so every
gather call / matmul schedule is a compile-time constant.  The multi-dim
best-fit packer fills tile capacity to ~97.7%, minimizing padded gather
indices (desc-gen cost is per-index, so padding is pure waste).
"""

import sys, os, types
sys.path.insert(0, "/opt/trn_rl_repo")

# antenv.axon_hooks shim (image's antenv stub lacks it); needed for NTFF trace.
if "antenv.axon_hooks" not in sys.modules:
    _hook = [None]
    _m = types.ModuleType("antenv.axon_hooks")
    _m.set_axon_ntff_profile_hook = lambda h: _hook.__setitem__(0, h)
    _m.get_axon_ntff_profile_hook = lambda: _hook[0]
    sys.modules["antenv.axon_hooks"] = _m
    try:
        import antenv
        antenv.axon_hooks = _m
        from trn_agent_boot.trn_boot import _ntff_profile_via_ctypes
        _m.set_axon_ntff_profile_hook(
            _ntff_profile_via_ctypes("/opt/axon/libaxon_pjrt.so"))
    except Exception:
        pass

import numpy as np
from dataclasses import dataclass

import concourse.bacc as bacc
import concourse.bass as bass
import concourse.mybir as mybir
import concourse.tile as tile
from concourse.bass import AP
from concourse.bass_utils import run_bass_kernel_spmd
from concourse.library_config import mlp

P = 128
H = 128
F_IN = 116
B = 2
ROW = 2 * P               # xg row: [b0 feats+oh | b1 feats+oh], 256 f16
NQ = 4                    # SWDGE queues (4 Q7 core pairs in parallel)


@dataclass(frozen=True)
class Cfg:
    n: int = 100000      # nodes
    ncores: int = 8
    tiles: int = 100     # dst tiles per core (128 slots each)
    chunks: int = 16     # 128-edge chunks per tile (4 per src chunk)
    group: int = 2       # tiles per gather-call group
    nsc: int = 4         # src chunks (int16 gather index reach)

    @property
    def ndst(self):
        return self.n // self.ncores

    @property
    def srcchunk(self):
        return -(-self.n // self.nsc)

    @property
    def npad(self):
        return -(-self.n // 512) * 512

    @property
    def cpt(self):        # chunks per (tile, src chunk)
        assert self.chunks % self.nsc == 0
        return self.chunks // self.nsc

    @property
    def ngroups(self):
        return self.tiles // self.group

    @property
    def call_chunks(self):  # chunks per gather call
        return self.group * self.cpt

    @property
    def call_idx(self):
        return self.call_chunks * P

    @property
    def ncalls(self):
        return self.ngroups * self.nsc

    @property
    def idxcols(self):
        return self.ncalls * (self.call_idx // 16)

    @property
    def nchunks_total(self):
        return self.tiles * self.chunks


CFG = Cfg()

f32 = mybir.dt.float32
f16 = mybir.dt.float16
i16 = mybir.dt.int16


def _build_program(cfg: Cfg):
    nc = bacc.Bacc("TRN2", num_swdge_queues=NQ)
    xg = nc.dram_tensor("xg", [cfg.npad, ROW], f16, kind="ExternalInput")
    w1e = nc.dram_tensor("w1e", [P, H], f16, kind="ExternalInput")
    idxt = nc.dram_tensor("idxt", [P, cfg.idxcols], i16, kind="ExternalInput")
    dlt = nc.dram_tensor("dlt", [P, cfg.nchunks_total], f16,
                         kind="ExternalInput")
    dcq = nc.dram_tensor("dcq", [P, cfg.tiles], f32, kind="ExternalInput")
    iot = nc.dram_tensor("iot", [P, P], f16, kind="ExternalInput")
    xgo = nc.dram_tensor("xgo", [P, cfg.tiles * ROW], f16,
                         kind="ExternalInput")
    accd = nc.dram_tensor("acc", [P, ROW], f32, kind="ExternalOutput")

    with tile.TileContext(nc) as tc:
        nc.gpsimd.load_library(mlp)
        with (
            tc.tile_pool(name="const", bufs=1) as cpool,
            tc.tile_pool(name="gat", bufs=12) as gpool,
            tc.tile_pool(name="oh", bufs=3) as ohpool,
            tc.tile_pool(name="agg", bufs=4) as apool,
            tc.tile_pool(name="x1c", bufs=4) as xpool,
            tc.tile_pool(name="psa", bufs=4, space="PSUM") as psa,
            tc.tile_pool(name="pso", bufs=4, space="PSUM") as pso,
        ):
            # constants / preloads: idx table first (gates the first gather),
            # split so early gathers start after the first slice lands.
            idx_sb = cpool.tile([P, cfg.idxcols], i16, tag="idx")
            nsplit = 8
            step = cfg.idxcols // nsplit
            for i in range(nsplit):
                nc.sync.dma_start(idx_sb[:, i * step:(i + 1) * step],
                                  idxt[:, i * step:(i + 1) * step])
            w1_sb = cpool.tile([P, H], f16, tag="w1")
            nc.sync.dma_start(w1_sb[:], w1e[:])
            iota_sb = cpool.tile([P, P], f16, tag="iota")
            nc.sync.dma_start(iota_sb[:], iot[:])
            dl_sb = cpool.tile([P, cfg.nchunks_total], f16, tag="dl")
            nc.sync.dma_start(dl_sb[:], dlt[:])
            dcq_sb = cpool.tile([P, cfg.tiles], f32, tag="dcq")
            nc.sync.dma_start(dcq_sb[:], dcq[:])
            xgo_sb = cpool.tile([P, cfg.tiles * ROW], f16, tag="xgo")
            # split so group 0's slice lands before its psum drain needs it
            xstep = cfg.tiles * ROW // 10
            for i in range(10):
                nc.sync.dma_start(xgo_sb[:, i * xstep:(i + 1) * xstep],
                                  xgo[:, i * xstep:(i + 1) * xstep])
            acc_sb = cpool.tile([P, ROW], f32, tag="acc")
            nc.vector.memset(acc_sb[:], 0)

            ic_per_call = cfg.call_idx // 16
            iota_ap = iota_sb[:, :]
            iota_3d = AP(iota_ap.tensor, iota_ap.offset,
                         [iota_ap.ap[0], (0, cfg.chunks), iota_ap.ap[1]])

            for g in range(cfg.ngroups):
                gts = []
                for s in range(cfg.nsc):
                    call = g * cfg.nsc + s
                    ic0 = call * ic_per_call
                    gt = gpool.tile([P, cfg.call_chunks, ROW], f16, tag="gt")
                    r0 = s * cfg.srcchunk
                    nc.gpsimd.dma_gather(
                        gt[:], xg[r0:r0 + cfg.srcchunk, :],
                        idx_sb[:, ic0:ic0 + ic_per_call],
                        cfg.call_idx, cfg.call_idx, ROW,
                        queue_num=s % NQ, single_packet=False)
                    gts.append(gt)
                for ti in range(cfg.group):
                    t = g * cfg.group + ti
                    # bulk one-hot build: oh_all[:, k, :] for the 16 chunks
                    # of tile t (k = s*cpt + j, host lays dlt out to match)
                    oh_all = ohpool.tile([P, cfg.chunks, P], f16, tag="oh")
                    dl_slice = dl_sb[:, t * cfg.chunks:(t + 1) * cfg.chunks]
                    nc.vector.tensor_tensor(
                        out=oh_all[:],
                        in0=dl_slice.to_broadcast([P, cfg.chunks, P]),
                        in1=iota_3d,
                        op=mybir.AluOpType.is_equal)
                    ps = psa.tile([P, ROW], f32, tag="ps", name=f"ps{t}")
                    start_mm = None
                    off = ti * cfg.cpt
                    for s in range(cfg.nsc):
                        for j in range(cfg.cpt):
                            k = s * cfg.cpt + j
                            last = (k == cfg.chunks - 1)
                            # aggT[feat, dst] += gathered_chunk.T @ onehot
                            # exactly one start=True matmul per psum tile
                            # (PSUM zero-regions are 2KB: start marks the
                            # whole region pending-zero -> unique + first)
                            mm0 = nc.tensor.matmul(
                                ps[:, 0:P],
                                lhsT=gts[s][:, off + j, 0:P],
                                rhs=oh_all[:, k, :],
                                start=(k == 0), stop=last)
                            mm1 = nc.tensor.matmul(
                                ps[:, P:ROW],
                                lhsT=gts[s][:, off + j, P:ROW],
                                rhs=oh_all[:, k, :],
                                start=False, stop=last)
                            if k == 0:
                                start_mm = mm0
                                bass._add_dep_helper(
                                    mm1.ins, start_mm.ins, sync=False,
                                    reason="half1 after psum start")
                            else:
                                for mm in (mm0, mm1):
                                    bass._add_dep_helper(
                                        mm.ins, start_mm.ins, sync=False,
                                        reason="accum after psum start")
                    # aggT += self-loop features (+ b1/dis row): dense add
                    agg_sb = apool.tile([P, ROW], f16, tag="agg")
                    nc.vector.tensor_tensor(
                        out=agg_sb[:], in0=ps[:],
                        in1=xgo_sb[:, t * ROW:(t + 1) * ROW],
                        op=mybir.AluOpType.add)
                    # out1[dst, h] = aggT.T @ W1ext  (per batch half)
                    po = pso.tile([P, ROW], f32, tag="po", name=f"po{t}")
                    wm0 = nc.tensor.matmul(
                        po[:, 0:P], lhsT=agg_sb[:, 0:P], rhs=w1_sb[:],
                        start=True, stop=True)
                    wm1 = nc.tensor.matmul(
                        po[:, P:ROW], lhsT=agg_sb[:, P:ROW], rhs=w1_sb[:],
                        start=False, stop=True)
                    bass._add_dep_helper(
                        wm1.ins, wm0.ins, sync=False,
                        reason="half1 after psum start")
                    # x1c = relu(out1 * dis*c)  (dis,c > 0 so scale commutes)
                    x1c = xpool.tile([P, ROW], f32, tag="x1c")
                    nc.scalar.activation(
                        out=x1c[:], in_=po[:],
                        func=mybir.ActivationFunctionType.Relu,
                        bias=0.0, scale=dcq_sb[:, t:t + 1])
                    nc.vector.tensor_tensor(
                        out=acc_sb[:], in0=acc_sb[:], in1=x1c[:],
                        op=mybir.AluOpType.add)

            nc.sync.dma_start(accd[:], acc_sb[:])

    nc.compile()
    return nc


_PROG_CACHE = {}


def _get_program(cfg: Cfg):
    if cfg not in _PROG_CACHE:
        _PROG_CACHE[cfg] = _build_program(cfg)
    return _PROG_CACHE[cfg]


def _pack_core(cfg: Cfg, core, src, dst):
    """Bin-pack this core's dst nodes into tiles; build gather/dstloc data.

    Returns (idx_w [128, idxcols] i16, dl2_w [128, nchunks] f16 in per-tile
    chunk order, tile_of, slot_of)."""
    n0 = core * cfg.ndst
    sel = (dst >= n0) & (dst < n0 + cfg.ndst)
    es = src[sel]
    ed = dst[sel]
    dl = ed - n0                       # local dst id
    sc = es // cfg.srcchunk            # src chunk of each edge

    cnt = np.bincount(dl * cfg.nsc + sc, minlength=cfg.ndst * cfg.nsc)
    cnt = cnt.reshape(cfg.ndst, cfg.nsc)

    caps = np.full((cfg.tiles, cfg.nsc), cfg.cpt * P, dtype=np.int64)
    for s in range(cfg.nsc):
        assert cnt[:, s].sum() <= caps[:, s].sum(), \
            f"core {core}: src chunk {s} demand exceeds capacity"

    # order by worst-dim demand; place to maximize the min remaining
    # per-src-chunk capacity (multi-dim balance packs ~98% full)
    order = np.argsort(-cnt.max(1), kind="stable")
    slots_used = np.zeros(cfg.tiles, dtype=np.int64)
    tile_of = np.full(cfg.ndst, -1, dtype=np.int64)
    slot_of = np.full(cfg.ndst, -1, dtype=np.int64)
    for nloc in order:
        need = cnt[nloc]
        ok = (caps >= need).all(axis=1) & (slots_used < P)
        if not ok.any():
            raise RuntimeError(f"core {core}: bin packing failed for node {nloc}")
        score = (caps - need).min(axis=1) * 1000 + (P - slots_used)
        score = np.where(ok, score, -1)
        t = int(np.argmax(score))
        tile_of[nloc] = t
        slot_of[nloc] = slots_used[t]
        slots_used[t] += 1
        caps[t] -= need

    # edge stream positions
    et = tile_of[dl]
    eslot = slot_of[dl]
    o = np.lexsort((sc, et))
    et_s, sc_s, slot_s, src_s = et[o], sc[o], eslot[o], es[o]
    ks = et_s * cfg.nsc + sc_s
    counts = np.bincount(ks, minlength=cfg.tiles * cfg.nsc)
    gbase = np.concatenate([[0], np.cumsum(counts)[:-1]])
    rank = np.arange(len(ks)) - gbase[ks]

    # stream base for (t, s): call (t//group, s), offset (t%group)*cpt chunks
    tt = np.arange(cfg.tiles)
    callno = (tt // cfg.group)[:, None] * cfg.nsc + np.arange(cfg.nsc)[None, :]
    pbase = callno * cfg.call_idx + (tt % cfg.group)[:, None] * cfg.cpt * P
    assert (counts.reshape(cfg.tiles, cfg.nsc) <= cfg.cpt * P).all()

    total = cfg.ncalls * cfg.call_idx
    idx_flat = np.zeros(total, dtype=np.int16)
    dl_flat = np.full(total, 255.0, dtype=np.float16)
    pos = pbase[et_s, sc_s] + rank
    idx_flat[pos] = (src_s - sc_s * cfg.srcchunk).astype(np.int16)
    dl_flat[pos] = slot_s.astype(np.float16)

    ci = cfg.call_idx
    idx_w = idx_flat.reshape(cfg.ncalls, ci // 16, 16).transpose(2, 0, 1)
    idx_w = np.tile(idx_w.reshape(16, -1), (8, 1))           # [128, idxcols]

    # dl in per-tile chunk order: column t*chunks + s*cpt + j holds the slot
    # ids of the chunk at stream call (t//group, s), chunk (t%group)*cpt + j.
    dl_st = dl_flat.reshape(cfg.ncalls, cfg.call_chunks, P)  # [call, chunk, P]
    dl2 = np.empty((cfg.tiles, cfg.chunks, P), dtype=np.float16)
    for ti in range(cfg.group):
        for s in range(cfg.nsc):
            # tiles with t%group==ti, their cpt chunks from call (g, s)
            dl2[ti::cfg.group, s * cfg.cpt:(s + 1) * cfg.cpt, :] = \
                dl_st[s::cfg.nsc, ti * cfg.cpt:(ti + 1) * cfg.cpt, :][
                    :cfg.ngroups]
    dl2_w = dl2.reshape(cfg.tiles * cfg.chunks, P).T.copy()  # [128, nchunks]
    return idx_w, dl2_w, tile_of, slot_of


def _prepare(cfg: Cfg, node, node_type, edge_index, embed, W1, b1):
    n = cfg.n
    src = edge_index[0].astype(np.int64)
    dst = edge_index[1].astype(np.int64)
    deg = (np.bincount(dst, minlength=n) + 1).astype(np.float64)
    dis64 = 1.0 / np.sqrt(deg)
    dis = dis64.astype(np.float32)
    s_arr = np.bincount(src, weights=dis64[dst], minlength=n)
    c = (dis64 * (s_arr + dis64)).astype(np.float32)
    dis_c = (dis64 * c.astype(np.float64)).astype(np.float32)

    T8 = embed.astype(np.float64) @ W1[F_IN:, :].astype(np.float64)
    w1e = np.zeros((P, H), dtype=np.float16)
    w1e[:F_IN] = W1[:F_IN]
    w1e[F_IN:F_IN + 8] = T8.astype(np.float16)
    w1e[F_IN + 8] = b1                      # b1 feature row (see xgo)

    # xg rows: [dis*node_b0 | dis*onehot | 0 pad] x2 halves
    xg = np.zeros((cfg.npad, ROW), dtype=np.float16)
    for b in range(B):
        xg[:n, b * P:b * P + F_IN] = node[b] * dis[:, None]
    oh_col = F_IN + node_type.astype(np.int64)          # one-hot position
    rows = np.arange(n)
    for b in range(B):
        xg[rows, b * P + oh_col] = dis
    # (row F_IN+8 stays 0 in xg: the b1 feature enters via xgo only)

    iota = np.tile(np.arange(P, dtype=np.float16), (P, 1))

    in_maps = []
    for core in range(cfg.ncores):
        idx_w, dl2_w, tile_of, slot_of = _pack_core(cfg, core, src, dst)
        n0 = core * cfg.ndst
        # xgo: per (tile, slot) the node's own xg row (self-loop term), with
        # the b1 feature slot set to 1/dis so out1 picks up b1/dis.
        nodes = np.arange(n0, n0 + cfg.ndst)
        xrows = xg[nodes].astype(np.float32)
        inv_dis = (1.0 / dis[nodes]).astype(np.float32)
        xrows[:, F_IN + 8] = inv_dis
        xrows[:, P + F_IN + 8] = inv_dis
        xgo = np.zeros((P, cfg.tiles * ROW), dtype=np.float16)
        # columns: t*ROW + half*P + slot
        for hf in range(2):
            colidx = tile_of * ROW + hf * P + slot_of
            xgo[:, colidx] = xrows[:, hf * P:(hf + 1) * P].T.astype(np.float16)
        dcq_w = np.zeros((P, cfg.tiles), dtype=np.float32)
        dcq_w[slot_of, tile_of] = dis_c[n0:n0 + cfg.ndst]
        m = {"xg": xg, "w1e": w1e, "idxt": idx_w, "dlt": dl2_w,
             "dcq": dcq_w, "iot": iota, "xgo": xgo}
        in_maps.append(m)
    return in_maps


def run(inputs, cfg: Cfg = CFG, trace: bool = False):
    node = np.asarray(inputs["node"], dtype=np.float32)
    node_type = np.asarray(inputs["node_type"])
    edge_index = np.asarray(inputs["edge_index"])
    embed = np.asarray(inputs["embed"], dtype=np.float32)
    W1 = np.asarray(inputs["W1"], dtype=np.float32)
    b1 = np.asarray(inputs["b1"], dtype=np.float32)
    W2 = np.asarray(inputs["W2"], dtype=np.float32)
    b2 = np.asarray(inputs["b2"], dtype=np.float32)

    in_maps = _prepare(cfg, node, node_type, edge_index, embed, W1, b1)
    nc = _get_program(cfg)
    res = run_bass_kernel_spmd(
        nc, in_maps, core_ids=list(range(cfg.ncores)), trace=trace,
        trace_cores=list(range(cfg.ncores)) if trace else None)

    total = np.zeros((B, H), dtype=np.float64)
    for core in range(cfg.ncores):
        acc = res.results[core]["acc"].astype(np.float64)   # [128, 2*H]
        total += acc.reshape(P, B, H).sum(axis=0)
    out = (total @ W2.astype(np.float64)) / cfg.n + b2.astype(np.float64)
    return out.astype(np.float32), res


def kernel(**inputs) -> np.ndarray:
    out, _ = run(inputs, CFG, trace=False)
    return out
